# revision 39
# baseline (speedup 1.0000x reference)
"""BertSelfAttention Trainium2 Bass kernel.

Full inputs in, full output out. Sharding: 8 cores = 4 batches x 2 head
groups (8 heads each). Per-core SPMD program (no collectives):

  phase A: cast-load X (fp32->bf16 SWDGE DMA), PE-transpose to X^T
           st-major so transposes pipeline right behind the cast DMAs
  phase B: cast-load W columns, project Q^T,K^T ([heads*64, S] layout,
           head dim on partitions) and V ([S, heads*64] + ones column)
  phase C: per head pair: S^T = K^T_h.T @ Q^T (row-packed, 2 heads per
           PE pass), exp(s/8 + mask) fused on ScalarE PSUM->SBUF bf16,
           ctx^T accum = [V|1].T @ P^T (row 64 = softmax denominators),
           PE-transpose ctx^T back, multiply by reciprocal row sums,
           assemble [128,512] out tiles, DMA to DRAM.
"""

import sys
from contextlib import ExitStack

import numpy as np

sys.path.insert(0, "/opt/trn_rl_repo")

import ml_dtypes  # noqa: E402

_np_bf16 = ml_dtypes.bfloat16

import concourse.bass as bass  # noqa: E402
from concourse import bacc  # noqa: E402
import concourse.mybir as mybir  # noqa: E402
import concourse.tile as tile  # noqa: E402

B, S, H = 4, 2048, 1024
NH, HD = 16, 64
GH = 8            # heads per core
GC = GH * HD      # 512 output cols per core
NP = 128          # partitions
NST = S // NP     # 16 s-tiles
NKH = H // NP     # 8 contraction tiles for projections
NQ = S // 512     # 4 q blocks of 512
NKT = S // NP     # 16 k tiles of 128
F32 = mybir.dt.float32
BF16 = mybir.dt.bfloat16
SCALE = 1.0 / 8.0  # 1/sqrt(HD)


def _emit(tc, x, wq, wk, wv, bqd, bkd, bvd, maskd, identd, identbd, out):
    nc = tc.nc
    with ExitStack() as ctx:
        const = ctx.enter_context(tc.tile_pool(name="const", bufs=1))
        big = ctx.enter_context(tc.tile_pool(name="big", bufs=1))

        ident_f = const.tile([NP, NP], F32, tag="identf")
        nc.sync.dma_start(out=ident_f[:], in_=identd)
        # touch Exp once so the ACT table set loads during the DMA-bound
        # prologue instead of on the first real softmax op
        warm = const.tile([1, 1], F32, tag="actwarm")
        nc.vector.memset(warm[:], 0.0)
        nc.scalar.activation(warm[:], warm[:], mybir.ActivationFunctionType.Exp)
        ident_bf = const.tile([NP, NP], BF16, tag="identbf")
        nc.sync.dma_start(out=ident_bf[:], in_=identbd)

        mask_sb = const.tile([NP, NKT], F32, tag="mask")
        nc.gpsimd.dma_start(out=mask_sb[:], in_=maskd.rearrange("(t p) -> p t", p=NP))
        bq_sb = const.tile([NP, 4], F32, tag="bq")
        nc.gpsimd.dma_start(out=bq_sb[:], in_=bqd.rearrange("(m p) -> p m", p=NP))
        bk_sb = const.tile([NP, 4], F32, tag="bk")
        nc.gpsimd.dma_start(out=bk_sb[:], in_=bkd.rearrange("(m p) -> p m", p=NP))
        bv_bc = const.tile([NP, GC], F32, tag="bvbc")
        nc.sync.dma_start(out=bv_bc[:], in_=bvd)

        # persistent SBUF tensors
        xt_all = big.tile([NP, NKH * S], BF16, tag="xtall", name="xtall")
        xt = [xt_all[:, i * S : (i + 1) * S] for i in range(NKH)]
        qt_sb = [big.tile([NP, S], BF16, tag=f"qt{i}", name=f"qt{i}") for i in range(4)]
        kt_sb = [big.tile([NP, S], BF16, tag=f"kt{i}", name=f"kt{i}") for i in range(4)]
        v_sb = [big.tile([NP, GH * (HD + 1)], BF16, tag=f"v{i}", name=f"v{i}") for i in range(NST)]
        wq_sb = big.tile([NP, NKH * GC], BF16, tag="wq")
        wk_sb = big.tile([NP, NKH * GC], BF16, tag="wk")
        wv_sb = big.tile([NP, NKH * GC], BF16, tag="wv")

        def load_w(wsb, wdr):
            nc.gpsimd.dma_start(
                out=wsb[:].rearrange("p (t c) -> p t c", t=NKH),
                in_=wdr.rearrange("(t p) c -> p t c", p=NP),
            )

        # ---- phase A: cast-load X chunks, PE-transpose st-major (each
        # s-chunk's 8 transposes land in one PSUM tile, one strided DVE
        # copy scatters it across the 8 X^T tiles) so transposes pipeline
        # right behind the cast DMAs ----
        with (
            tc.tile_pool(name="xbfp", bufs=1) as xbfp,
            tc.tile_pool(name="psA", bufs=6, space="PSUM") as psA,
        ):
            xbf = [
                xbfp.tile([NP, H], BF16, tag=f"xbf{i}", name=f"xbf{i}")
                for i in range(NST)
            ]
            # DMA order matters: X chunks + Wv feed the earliest PE work;
            # Wq/Wk aren't needed until the pair-0 projections (~15us in)
            for st in range(NST // 2):
                nc.gpsimd.dma_start(out=xbf[st][:], in_=x[st * NP : (st + 1) * NP, :])
            load_w(wv_sb, wv)
            for st in range(NST // 2, NST):
                nc.gpsimd.dma_start(out=xbf[st][:], in_=x[st * NP : (st + 1) * NP, :])
            load_w(wk_sb, wk)
            load_w(wq_sb, wq)
            for st in range(NST):
                pa = psA.tile([NP, H], BF16, tag="pa", name="pa")  # 1 bank
                for ht in range(NKH):
                    nc.tensor.transpose(
                        pa[:, ht * NP : (ht + 1) * NP],
                        xbf[st][:, ht * NP : (ht + 1) * NP],
                        ident_bf[:],
                    )
                # xt[ht][:, st*NP:(st+1)*NP] <- pa[:, ht*NP:(ht+1)*NP] for all ht
                nc.vector.tensor_copy(
                    xt_all[:].rearrange("p (h s) -> p h s", h=NKH)[
                        :, :, st * NP : (st + 1) * NP
                    ],
                    pa[:].rearrange("p (h s) -> p h s", h=NKH),
                )

        # ---- phase B+C interleaved: V first, then per head pair:
        # K^T/Q^T projection for that pair followed by its attention, so
        # remaining projections fill PE slack in the ACT-bound k-loop ----
        def proj_v_units(psB, st):
            cell = {}

            def mm(kt, cell=cell):
                if kt == 0:
                    cell["pv"] = psB.tile([NP, 512], F32, tag="proj", name="pv")
                nc.tensor.matmul(
                    cell["pv"][:],
                    xt[kt][:, st * NP : (st + 1) * NP],
                    wv_sb[:, kt * GC : (kt + 1) * GC],
                    start=(kt == 0),
                    stop=(kt == NKH - 1),
                )

            def drain(cell=cell, st=st):
                # ones in every column once; V values overwrite cols 0..63 of
                # each 65-block, leaving col 64 = 1.0 (softmax denominators)
                nc.vector.memset(v_sb[st][:], 1.0)
                v3 = v_sb[st][:].rearrange("p (h e) -> p h e", e=HD + 1)
                nc.vector.tensor_tensor(
                    out=v3[:, :, 0:HD],
                    in0=cell["pv"][:].rearrange("p (h e) -> p h e", e=HD),
                    in1=bv_bc[:].rearrange("p (h e) -> p h e", e=HD),
                    op=mybir.AluOpType.add,
                )

            return [(lambda kt=kt, mm=mm: mm(kt)) for kt in range(NKH)] + [drain]

        def proj_qk_units(psB, mt):
            """Emission closures for pair mt's K^T/Q^T projections, one PE
            matmul (or one drain) per unit, so they can be sprinkled into
            the previous pair's ACT-bound attention loop."""
            units = []
            for wsb, dst, bias in ((wk_sb, kt_sb, bk_sb), (wq_sb, qt_sb, bq_sb)):
                for nt in range(4):
                    cell = {}

                    def mm(kt, wsb=wsb, nt=nt, cell=cell):
                        if kt == 0:
                            cell["pp"] = psB.tile(
                                [NP, 512], F32, tag="proj", name="pp"
                            )
                        nc.tensor.matmul(
                            cell["pp"][:],
                            wsb[:, kt * GC + mt * NP : kt * GC + (mt + 1) * NP],
                            xt[kt][:, nt * 512 : (nt + 1) * 512],
                            start=(kt == 0),
                            stop=(kt == NKH - 1),
                        )

                    def drain(dst=dst, bias=bias, nt=nt, cell=cell):
                        nc.vector.tensor_scalar_add(
                            dst[mt][:, nt * 512 : (nt + 1) * 512],
                            cell["pp"][:],
                            bias[:, mt : mt + 1],
                        )

                    units.extend([(lambda kt=kt, mm=mm: mm(kt)) for kt in range(NKH)])
                    units.append(drain)
            return units

        with (
            tc.tile_pool(name="psB", bufs=2, space="PSUM") as psB,
            tc.tile_pool(name="psS", bufs=2, space="PSUM") as psS,
            tc.tile_pool(name="psC", bufs=2, space="PSUM") as psC,
            tc.tile_pool(name="ppool", bufs=6) as ppool,
            tc.tile_pool(name="cspool", bufs=6) as cspool,
            tc.tile_pool(name="obpool", bufs=1) as obpool,
            tc.tile_pool(name="rcpool", bufs=8) as rcpool,
        ):
            obs = {
                (qt, cj): obpool.tile([NP, GC], F32, tag=f"ob{qt}{cj}", name="ob")
                for qt in range(NQ)
                for cj in range(4)
            }
            # pair-0 K fully + Q first q-block upfront; V and the rest of
            # pair-0 Q interleave into the first q-block's k-loop so scores
            # (and ScalarE) start as early as possible
            u0 = proj_qk_units(psB, 0)
            for u in u0[: 4 * (NKH + 1) + (NKH + 1)]:
                u()
            q0_rest = list(u0[4 * (NKH + 1) + (NKH + 1) :])
            # pair-0 Q block nt is first read by q-block nt: defer each to
            # the preceding q-block instead of piling all into q-block 0
            q0_groups = [
                q0_rest[i * (NKH + 1) : (i + 1) * (NKH + 1)] for i in range(3)
            ]
            v_groups = [proj_v_units(psB, st) for st in range(NST)]
            for g in v_groups[:2]:
                for u in g:
                    u()
            for pr in range(4):
                h0, h1 = 2 * pr, 2 * pr + 1
                pending = proj_qk_units(psB, pr + 1) if pr < 3 else []
                iters_left = (3 if pr == 0 else NQ) * NKT
                for qt in range(NQ):
                    cps = [
                        psC.tile([HD + 1, 512], F32, tag="ctx", name="ctx0"),
                        psC.tile([HD + 1, 512], F32, tag="ctx", name="ctx1"),
                    ]

                    def emit_ctx(kt, pt):
                        for i, hh in enumerate((h0, h1)):
                            v3 = v_sb[kt][:].rearrange("p (h e) -> p h e", e=HD + 1)
                            nc.tensor.matmul(
                                cps[i][:],
                                v3[:, hh, :],
                                pt[:, i * 512 : (i + 1) * 512],
                                start=(kt == 0),
                                stop=(kt == NKT - 1),
                            )

                    prev = None
                    for kt in range(NKT):
                        sc = psS.tile([NP, 1024], F32, tag="sc", name="sc")
                        for i in range(2):
                            nc.tensor.matmul(
                                sc[:, i * 512 : (i + 1) * 512],
                                kt_sb[pr][i * 64 : (i + 1) * 64,
                                          kt * NP : (kt + 1) * NP],
                                qt_sb[pr][i * 64 : (i + 1) * 64,
                                          qt * 512 : (qt + 1) * 512],
                                start=True,
                                stop=True,
                                tile_position=(i * 64, 0),
                            )
                        if prev is not None:
                            emit_ctx(*prev)
                        pt = ppool.tile([NP, 1024], BF16, tag="pt", name="pt")
                        nc.scalar.activation(
                            pt[:],
                            sc[:],
                            mybir.ActivationFunctionType.Exp,
                            bias=mask_sb[:, kt : kt + 1],
                            scale=SCALE,
                        )
                        prev = (kt, pt)
                        # fill PE slack: during the first q-block, stream in V
                        # and the rest of pair-0 Q; afterwards the next pair's
                        # K/Q projection work
                        if pr == 0 and qt == 0 and kt + 2 < NST:
                            for u in v_groups[kt + 2]:
                                u()
                        if pr == 0 and qt < 3:
                            g = q0_groups[qt]
                            n_emit = -(-len(g) // (NKT - kt)) if g else 0
                            for _ in range(n_emit):
                                g.pop(0)()
                        if not (pr == 0 and qt == 0):
                            n_emit = -(-len(pending) // iters_left) if pending else 0
                            for _ in range(n_emit):
                                pending.pop(0)()
                            iters_left -= 1
                    emit_ctx(*prev)

                    for i, hh in enumerate((h0, h1)):
                        cs = cspool.tile([HD + 1, 512], F32, tag="cs", name="cs")
                        nc.vector.tensor_copy(cs[:], cps[i][:])
                        for cj in range(4):
                            tp = psC.tile([NP, HD + 1], F32, tag="ctx", name="tp")
                            nc.tensor.transpose(
                                tp[:],
                                cs[:, cj * NP : (cj + 1) * NP],
                                ident_f[0 : HD + 1, 0 : HD + 1],
                            )
                            rc = rcpool.tile([NP, 1], F32, tag="rc", name="rc")
                            nc.vector.reciprocal(rc[:], tp[:, HD : HD + 1])
                            nc.vector.tensor_scalar_mul(
                                obs[(qt, cj)][:, hh * HD : (hh + 1) * HD],
                                tp[:, 0:HD],
                                rc[:],
                            )
                    if pr == 3:
                        for cj in range(4):
                            r0 = qt * 512 + cj * NP
                            nc.sync.dma_start(
                                out=out[r0 : r0 + NP, :], in_=obs[(qt, cj)][:]
                            )


_NC_CACHE = {}


def _get_nc():
    if "nc" not in _NC_CACHE:
        nc = bacc.Bacc("TRN2", target_bir_lowering=False, debug=False, enable_asserts=False)
        x = nc.dram_tensor("x", [S, H], F32, kind="ExternalInput").ap()
        wq = nc.dram_tensor("wq", [H, GC], F32, kind="ExternalInput").ap()
        wk = nc.dram_tensor("wk", [H, GC], F32, kind="ExternalInput").ap()
        wv = nc.dram_tensor("wv", [H, GC], F32, kind="ExternalInput").ap()
        bq = nc.dram_tensor("bq", [GC], F32, kind="ExternalInput").ap()
        bk = nc.dram_tensor("bk", [GC], F32, kind="ExternalInput").ap()
        bv = nc.dram_tensor("bv", [NP, GC], F32, kind="ExternalInput").ap()
        mask = nc.dram_tensor("mask", [S], F32, kind="ExternalInput").ap()
        ident = nc.dram_tensor("ident", [NP, NP], F32, kind="ExternalInput").ap()
        identb = nc.dram_tensor("identb", [NP, NP], BF16, kind="ExternalInput").ap()
        out = nc.dram_tensor("out", [S, GC], F32, kind="ExternalOutput").ap()
        with tile.TileContext(nc) as tc:
            _emit(tc, x, wq, wk, wv, bq, bk, bv, mask, ident, identb, out)
        nc.compile()
        _NC_CACHE["nc"] = nc
    return _NC_CACHE["nc"]


def _in_maps(inputs):
    hs = np.ascontiguousarray(np.asarray(inputs["hidden_states"], np.float32))
    am = np.asarray(inputs["attention_mask"], np.float32)
    ws = {
        k: np.asarray(inputs[k], np.float32) for k in ("Wq", "Wk", "Wv")
    }
    bs = {k: np.asarray(inputs[k], np.float32) for k in ("bq", "bk", "bv")}
    maps = []
    for c in range(8):
        b, g = c // 2, c % 2
        cols = slice(g * GC, (g + 1) * GC)
        maps.append(
            {
                "x": np.ascontiguousarray(hs[b]),
                "wq": np.ascontiguousarray(ws["Wq"][:, cols]),
                "wk": np.ascontiguousarray(ws["Wk"][:, cols]),
                "wv": np.ascontiguousarray(ws["Wv"][:, cols]),
                "bq": np.ascontiguousarray(bs["bq"][cols]),
                "bk": np.ascontiguousarray(bs["bk"][cols]),
                "bv": np.ascontiguousarray(np.broadcast_to(bs["bv"][cols], (NP, GC))),
                "mask": np.ascontiguousarray(am[b, 0, 0, :]),
                "ident": np.eye(NP, dtype=np.float32),
                "identb": np.eye(NP, dtype=_np_bf16),
            }
        )
    return maps


class _Runner:
    """Cached PJRT executor for the SPMD bass program (8 cores).

    Mirrors concourse.bass2jax.run_bass_via_pjrt but keeps the jitted
    shard_map executable alive across calls so the NEFF compiles once.
    """

    def __init__(self, nc, n_cores=8):
        import jax
        from jax.experimental.shard_map import shard_map
        from jax.sharding import Mesh, PartitionSpec

        from concourse import bass2jax, mybir as _mybir

        bass2jax.install_neuronx_cc_hook()
        self.jax = jax
        self.nc = nc
        self.n_cores = n_cores
        assert nc.dbg_addr is None
        part_name = (
            nc.partition_id_tensor.name if nc.partition_id_tensor is not None else None
        )

        in_names, out_names, out_avals, zero_outs = [], [], [], []
        for alloc in nc.m.functions[0].allocations:
            if not isinstance(alloc, _mybir.MemoryLocationSet):
                continue
            name = alloc.memorylocations[0].name
            if alloc.kind == "ExternalInput":
                if name != part_name:
                    in_names.append(name)
            elif alloc.kind == "ExternalOutput":
                out_names.append(name)
                shape = tuple(alloc.tensor_shape)
                dtype = _mybir.dt.np(alloc.dtype)
                out_avals.append(jax.core.ShapedArray(shape, dtype))
                zero_outs.append(np.zeros(shape, dtype))
        self.in_names = list(in_names)
        self.out_names = list(out_names)
        self.out_avals = out_avals
        self.zero_outs = zero_outs
        n_params, n_outs = len(in_names), len(out_avals)
        all_names = in_names + out_names
        if part_name is not None:
            all_names = all_names + [part_name]
        donate = tuple(range(n_params, n_params + n_outs))

        def _body(*args):
            operands = list(args)
            if part_name is not None:
                operands.append(bass2jax.partition_id_tensor())
            outs = bass2jax._bass_exec_p.bind(
                *operands,
                out_avals=tuple(out_avals),
                in_names=tuple(all_names),
                out_names=tuple(out_names),
                lowering_input_output_aliases=(),
                sim_require_finite=True,
                sim_require_nnan=True,
                nc=nc,
            )
            return tuple(outs)

        self._body = _body
        devices = jax.devices()[:n_cores]
        self.mesh = Mesh(np.asarray(devices), ("core",))
        self.pspec = PartitionSpec("core")
        in_specs = (self.pspec,) * (n_params + n_outs)
        out_specs = (self.pspec,) * n_outs
        self.sharded = jax.jit(
            shard_map(
                _body,
                mesh=self.mesh,
                in_specs=in_specs,
                out_specs=out_specs,
                check_rep=False,
            ),
            donate_argnums=donate,
            keep_unused=True,
        )

    def make_chain(self, n):
        """jit of n chained executions (each feeding its outputs as the next
        call's donated output buffers) — serialized by data deps, one dispatch."""
        import jax
        from jax.experimental.shard_map import shard_map

        n_params = len(self.in_names)
        n_outs = len(self.out_names)

        def _chain(*args):
            ins = list(args[:n_params])
            cur = tuple(args[n_params:])
            if n == 1:
                return self._body(*ins, *cur)
            return jax.lax.fori_loop(
                0, n, lambda i, c: self._body(*ins, *c), cur, unroll=False
            )

        in_specs = (self.pspec,) * (n_params + n_outs)
        out_specs = (self.pspec,) * n_outs
        return jax.jit(
            shard_map(
                _chain,
                mesh=self.mesh,
                in_specs=in_specs,
                out_specs=out_specs,
                check_rep=False,
            ),
            donate_argnums=tuple(range(n_params, n_params + n_outs)),
            keep_unused=True,
        )

    def concat_inputs(self, in_maps):
        return [
            np.concatenate([np.asarray(m[name]) for m in in_maps], axis=0)
            for name in self.in_names
        ]

    def fresh_zeros(self):
        return [
            np.zeros((self.n_cores * z.shape[0], *z.shape[1:]), z.dtype)
            for z in self.zero_outs
        ]

    def __call__(self, in_maps):
        out_arrs = self.sharded(*self.concat_inputs(in_maps), *self.fresh_zeros())
        return [
            {
                name: np.asarray(out_arrs[i]).reshape(
                    self.n_cores, *self.out_avals[i].shape
                )[c]
                for i, name in enumerate(self.out_names)
            }
            for c in range(self.n_cores)
        ]


def _get_runner():
    if "runner" not in _NC_CACHE:
        _NC_CACHE["runner"] = _Runner(_get_nc())
    return _NC_CACHE["runner"]


def _assemble(results):
    full = np.empty((B, S, H), np.float32)
    for c in range(8):
        b, g = c // 2, c % 2
        full[b, :, g * GC : (g + 1) * GC] = results[c]["out"]
    return full


def _run(inputs, trace=False, **kwargs):
    if trace:
        from concourse.bass_utils import run_bass_kernel_spmd

        nc = _get_nc()
        res = run_bass_kernel_spmd(
            nc, _in_maps(inputs), core_ids=list(range(8)), trace=True, **kwargs
        )
        return _assemble(res.results), res

    return _assemble(_get_runner()(_in_maps(inputs))), None


def kernel(**inputs):
    return _run(inputs)[0]
